# revision 4
# baseline (speedup 1.0000x reference)
"""Trainium2 Bass kernel for a pre-LN causal transformer block (v3).

Sharding: data-parallel over batch. B=16 split across 8 NeuronCores
(2 sequences per core); weights replicated. No collectives needed.

v3 device changes vs the baseline kernel:
  - all matmuls in bf16 (fp32 PSUM accumulation): same PE rate as f32r
    but half the SBUF/DMA footprint;
  - weights are uploaded pre-cast to bf16, x/out cross the host link in
    bf16 (tolerance is 2e-2; measured end-to-end error ~2e-3);
  - FFN up/down fused per F-chunk with the up activations kept in SBUF
    (the old kernel spilled 33MB of upT to DRAM and read it back);
  - ctxT kept in SBUF (old kernel round-tripped it through DRAM);
  - far fewer, larger weight DMAs with deep prefetch pools (the old
    kernel's 4-matmul bursts stalled ~2-3us on every 128KB weight DMA);
  - PE transposes packed 4-per-PSUM-bank, drains split ACT/DVE.

Host path: module compiled once per process, jitted PJRT callable cached,
weights device_put replicated once (keyed by caller array identity);
steady-state calls move only x (bf16) in and out (bf16) back.
"""

import os
import sys

import numpy as np

for _p in ("/opt/trn_rl_repo", "/root/.axon_site/_ro/trn_rl_repo"):
    if os.path.isdir(_p) and _p not in sys.path:
        sys.path.append(_p)

import concourse.bass as bass
import concourse.tile as tile
from concourse import mybir
from concourse._compat import axon_active
from concourse.bass_utils import run_bass_kernel_spmd
from concourse.masks import make_identity

F32 = mybir.dt.float32
BF16 = mybir.dt.bfloat16
AX = mybir.AxisListType.X
AF = mybir.ActivationFunctionType
ALU = mybir.AluOpType

B, T, D, H = 16, 512, 2048, 4
HD = D // H            # 512
F = 4 * D              # 8192
N_CORES = 8
BPC = B // N_CORES     # 2 sequences per core
NT = BPC * T           # 1024 tokens per core
EPS = 1e-5
NEG = -1.0e30

DT = D // 128          # 16 d-tiles
RT = NT // 128         # 8 token-row tiles
KT = HD // 128         # 4 head-dim tiles
ST = T // 128          # 4 seq tiles per sequence
FT = F // 128          # 64 ffn tiles
FCH = 8                # ffn f-chunks
FCT = FT // FCH        # 8 f-tiles per chunk

_MAXW = 1

WEIGHT_NAMES = (
    "Wq", "Wk", "Wv", "Wo", "bo", "W1", "b1", "W2", "b2",
    "g1", "be1", "g2", "be2",
)
BF16_WEIGHTS = {"Wq", "Wk", "Wv", "Wo", "W1", "W2"}


def _split_sync_waits(nc):
    """Hoist excess semaphore waits into standalone same-engine drains."""
    for f in nc.m.functions:
        for blk in f.blocks:
            newl = []
            for inst in blk.instructions:
                si = inst.sync_info
                if si is not None and si.on_wait and len(si.on_wait) > _MAXW:
                    waits = list(si.on_wait)
                    extra, keep = waits[:-_MAXW], waits[-_MAXW:]
                    for i in range(0, len(extra), _MAXW):
                        d = mybir.InstDrain(
                            name=f"{inst.name}-wsplit{i}", ins=[], outs=[]
                        )
                        d.engine = inst.engine
                        d.sync_info = mybir.SyncInfo(
                            on_wait=extra[i : i + _MAXW], on_update=[]
                        )
                        newl.append(d)
                    inst.sync_info = mybir.SyncInfo(
                        on_wait=keep, on_update=list(si.on_update)
                    )
                newl.append(inst)
            blk.instructions = newl


def _layernorm_rows(ctx, src_ap, dst_tile, g_b=None, be_b=None):
    """dst(bf16) = LN(src) * g + be for one [128, D] row tile."""
    nc, small = ctx["nc"], ctx["small"]
    eps_sb = ctx["eps_sb"]
    stats = small.tile([128, 4, 6], F32, tag="stats")
    xr = src_ap.rearrange("p (c f) -> p c f", c=4)
    for c in range(4):
        nc.vector.bn_stats(out=stats[:, c, :], in_=xr[:, c, :])
    mv = small.tile([128, 2], F32, tag="mv")
    nc.vector.bn_aggr(out=mv, in_=stats)
    rstd = small.tile([128, 1], F32, tag="rstd")
    nc.scalar.activation(
        out=rstd, in_=mv[:, 1:2], func=AF.Sqrt, bias=eps_sb, scale=1.0
    )
    nc.vector.reciprocal(out=rstd, in_=rstd)
    if g_b is None and be_b is None:
        nc.vector.tensor_scalar(
            out=dst_tile, in0=src_ap, scalar1=mv[:, 0:1], scalar2=rstd,
            op0=ALU.subtract, op1=ALU.mult,
        )
        return
    tmp = ctx["lnt"].tile([128, D], F32, tag="lntmp")
    nc.vector.tensor_scalar(
        out=tmp, in0=src_ap, scalar1=mv[:, 0:1], scalar2=rstd,
        op0=ALU.subtract, op1=ALU.mult,
    )
    if g_b is not None:
        nc.vector.tensor_mul(out=tmp, in0=tmp, in1=g_b)
    if be_b is not None:
        nc.vector.tensor_add(out=dst_tile, in0=tmp, in1=be_b)
    else:
        nc.vector.tensor_copy(out=dst_tile, in_=tmp)


def _transpose_rows(ctx, src_tile, dst_tiles, r):
    """PE-transpose [128, n_c*128] bf16 row tile r into dst_tiles[c][:, r-slice].

    Transposes are packed 4 per PSUM bank; drains alternate ACT/DVE."""
    nc, ps, ident = ctx["nc"], ctx["ps"], ctx["ident"]
    n_c = src_tile.shape[-1] // 128
    for g in range((n_c + 3) // 4):
        cs = list(range(g * 4, min(g * 4 + 4, n_c)))
        pt = ps.tile([128, 512], BF16, tag="ps")
        for i, c in enumerate(cs):
            nc.tensor.transpose(
                pt[:, i * 128 : (i + 1) * 128],
                src_tile[:, c * 128 : (c + 1) * 128],
                ident,
            )
        for i, c in enumerate(cs):
            dst = dst_tiles[c][:, r * 128 : (r + 1) * 128]
            src = pt[:, i * 128 : (i + 1) * 128]
            if c % 2 == 0:
                nc.scalar.copy(out=dst, in_=src)
            else:
                nc.vector.tensor_copy(out=dst, in_=src)


def _bcast_vec(ctx, dram_vec):
    nc = ctx["nc"]
    t = ctx["bcast"].tile([128, D], F32, tag="bc")
    nc.sync.dma_start(out=t, in_=dram_vec[None, :].to_broadcast((128, D)))
    return t


def _phase_a(ctx, g1_id):
    """LN1(x) + PE-transpose -> hT [D, NT] bf16. Returns hT tiles."""
    nc, tc = ctx["nc"], ctx["tc"]
    g1_b = be1_b = None
    if not g1_id:
        g1_b = _bcast_vec(ctx, ctx["g1"])
        be1_b = _bcast_vec(ctx, ctx["be1"])
    feat = ctx["feat"]
    stage = ctx["stage"]
    hT = [feat.tile([128, NT], BF16, tag="feat", name=f"hT{c}") for c in range(DT)]
    x_flat = ctx["x_flat"]
    for r in range(RT):
        x_t = stage.tile([128, D], BF16, tag="xstage")
        nc.sync.dma_start(out=x_t, in_=x_flat[r * 128 : (r + 1) * 128, :])
        h_t = stage.tile([128, D], BF16, tag="hstage")
        _layernorm_rows(ctx, x_t, h_t, g1_b, be1_b)
        _transpose_rows(ctx, h_t, hT, r)
    return hT


def _attn_head(ctx, h, hT):
    """One attention head: QKV projections, causal softmax, PV -> ctxT."""
    nc, ps = ctx["nc"], ctx["ps"]
    ident, mask, small = ctx["ident"], ctx["mask"], ctx["small"]
    ctxT = ctx["ctxT"]
    scale = float(HD) ** -0.5

    wq_t = [ctx["wqp"].tile([128, 1024], BF16, tag="wq", name=f"wq{h}_{g}")
            for g in range(8)]
    wk_t = [ctx["wkp"].tile([128, 1024], BF16, tag="wk", name=f"wk{h}_{g}")
            for g in range(8)]
    wv_t = [ctx["wvp"].tile([128, 1024], BF16, tag="wv", name=f"wv{h}_{g}")
            for g in range(8)]
    for g in range(8):
        for wt, wsrc in ((wq_t, ctx["Wq"]), (wk_t, ctx["Wk"]), (wv_t, ctx["Wv"])):
            nc.sync.dma_start(
                out=wt[g].rearrange("p (t c) -> p t c", t=2),
                in_=wsrc[h, g * 256 : (g + 1) * 256, :].rearrange(
                    "(t p) c -> p t c", p=128
                ),
            )

    qT = [ctx["qp"].tile([128, NT], BF16, tag="qp", name=f"qT{h}_{c}")
          for c in range(KT)]
    kT = [ctx["kp"].tile([128, NT], BF16, tag="kp", name=f"kT{h}_{c}")
          for c in range(KT)]
    v_sb = [ctx["vp"].tile([128, HD], BF16, tag="vp", name=f"v{h}_{c}")
            for c in range(BPC * ST)]

    # Q and K: out[hd, t] = W.T @ h ; 4 chains (kth2, b) per half
    for proj, w_t, dsts in (("q", wq_t, qT), ("k", wk_t, kT)):
        for half in range(2):
            acc = [
                ps.tile([128, 512], F32, tag="ps", name=f"{proj}acc{h}_{half}_{i}")
                for i in range(4)
            ]
            for d in range(DT):
                wd = w_t[d // 2][:, (d % 2) * 512 : (d % 2) * 512 + 512]
                for kth2 in range(2):
                    kth = half * 2 + kth2
                    for b in range(BPC):
                        nc.tensor.matmul(
                            acc[kth2 * 2 + b],
                            wd[:, kth * 128 : (kth + 1) * 128],
                            hT[d][:, b * T : (b + 1) * T],
                            start=(d == 0),
                            stop=(d == DT - 1),
                        )
            for kth2 in range(2):
                kth = half * 2 + kth2
                for b in range(BPC):
                    dst = dsts[kth][:, b * T : (b + 1) * T]
                    if proj == "q":
                        nc.scalar.mul(out=dst, in_=acc[kth2 * 2 + b], mul=scale)
                    else:
                        nc.vector.tensor_copy(out=dst, in_=acc[kth2 * 2 + b])

    # V: out[t, hd] = h @ Wv ; 4 chains (st) per b
    for b in range(BPC):
        acc = [
            ps.tile([128, 512], F32, tag="ps", name=f"vacc{h}_{b}_{i}")
            for i in range(ST)
        ]
        for d in range(DT):
            wd = wv_t[d // 2][:, (d % 2) * 512 : (d % 2) * 512 + 512]
            for st in range(ST):
                nc.tensor.matmul(
                    acc[st],
                    hT[d][:, b * T + st * 128 : b * T + (st + 1) * 128],
                    wd,
                    start=(d == 0),
                    stop=(d == DT - 1),
                )
        for st in range(ST):
            nc.vector.tensor_copy(out=v_sb[b * ST + st], in_=acc[st])

    # causal softmax + PV per sequence
    for b in range(BPC):
        attnT = [
            ctx["attntp"].tile([128, T], BF16, tag="attntp", name=f"attnT{h}_{b}_{c}")
            for c in range(ST)
        ]
        for tt in range(ST):
            ns = 128 * (tt + 1)
            ps_s = ps.tile([128, 512], F32, tag="ps")
            for kt in range(KT):
                nc.tensor.matmul(
                    ps_s[:, :ns],
                    qT[kt][:, b * T + tt * 128 : b * T + (tt + 1) * 128],
                    kT[kt][:, b * T : b * T + ns],
                    start=(kt == 0),
                    stop=(kt == KT - 1),
                )
            nc.vector.tensor_add(
                out=ps_s[:, tt * 128 : ns], in0=ps_s[:, tt * 128 : ns], in1=mask
            )
            negmax = small.tile([128, 1], F32, tag="negmax")
            nc.vector.reduce_max(out=negmax, in_=ps_s[:, :ns], axis=AX, negate=True)
            attn_t = ctx["attnp"].tile([128, T], BF16, tag="attnp")
            rowsum = small.tile([128, 1], F32, tag="rowsum")
            nc.scalar.activation(
                out=attn_t[:, :ns], in_=ps_s[:, :ns], func=AF.Exp,
                bias=negmax, scale=1.0, accum_out=rowsum,
            )
            rinv = small.tile([128, 1], F32, tag="rinv")
            nc.vector.reciprocal(out=rinv, in_=rowsum)
            nc.vector.tensor_scalar_mul(
                out=attn_t[:, :ns], in0=attn_t[:, :ns], scalar1=rinv
            )
            pt = ps.tile([128, 512], BF16, tag="ps")
            for st in range(tt + 1):
                nc.tensor.transpose(
                    pt[:, st * 128 : (st + 1) * 128],
                    attn_t[:, st * 128 : (st + 1) * 128],
                    ident,
                )
            for st in range(tt + 1):
                dst = attnT[st][:, tt * 128 : (tt + 1) * 128]
                src = pt[:, st * 128 : (st + 1) * 128]
                if st % 2 == 0:
                    nc.scalar.copy(out=dst, in_=src)
                else:
                    nc.vector.tensor_copy(out=dst, in_=src)
        for ht in range(KT):
            ps_c = ps.tile([128, 512], F32, tag="ps")
            for st in range(ST):
                # causal: s-tile st only feeds outputs t >= st*128
                nc.tensor.matmul(
                    ps_c[:, st * 128 : T],
                    v_sb[b * ST + st][:, ht * 128 : (ht + 1) * 128],
                    attnT[st][:, st * 128 : T],
                    start=(st == 0),
                    stop=(st == ST - 1),
                    skip_group_check=True,
                )
            nc.vector.tensor_copy(
                out=ctxT[h * KT + ht][:, b * T : (b + 1) * T], in_=ps_c
            )


def _phase_ab(ctx, g1_id):
    tc = ctx["tc"]
    with (
        tc.tile_pool(name="feat", bufs=DT)
    ) as feat, (
        tc.tile_pool(name="stage", bufs=2)
    ) as stage, (
        tc.tile_pool(name="lnt", bufs=1)
    ) as lnt, (
        tc.tile_pool(name="wqp", bufs=8)
    ) as wqp, (
        tc.tile_pool(name="wkp", bufs=8)
    ) as wkp, (
        tc.tile_pool(name="wvp", bufs=8)
    ) as wvp, (
        tc.tile_pool(name="qp", bufs=6)
    ) as qp, (
        tc.tile_pool(name="kp", bufs=6)
    ) as kp, (
        tc.tile_pool(name="vp", bufs=12)
    ) as vp, (
        tc.tile_pool(name="attnp", bufs=3)
    ) as attnp, (
        tc.tile_pool(name="attntp", bufs=8)
    ) as attntp:
        ctx.update(
            feat=feat, stage=stage, lnt=lnt, wqp=wqp, wkp=wkp, wvp=wvp,
            qp=qp, kp=kp, vp=vp, attnp=attnp, attntp=attntp,
        )
        hT = _phase_a(ctx, g1_id)
        for h in range(H):
            _attn_head(ctx, h, hT)


def _phase_wo(ctx, bo_zero):
    """x2 = x + ctxT.T @ Wo (+bo), f32, SBUF-resident."""
    nc, tc, ps = ctx["nc"], ctx["tc"], ctx["ps"]
    ctxT, x_flat = ctx["ctxT"], ctx["x_flat"]
    bo_b = None if bo_zero else _bcast_vec(ctx, ctx["bo"])
    x2 = ctx["x2"]
    with (
        tc.tile_pool(name="wop", bufs=24)
    ) as wop, (
        tc.tile_pool(name="xre", bufs=3)
    ) as xre:
        for half in range(2):
            wo_res = [
                wop.tile([128, 1024], BF16, tag="wop", name=f"wo{half}_{c}")
                for c in range(DT)
            ]
            for c in range(DT):
                nc.sync.dma_start(
                    out=wo_res[c],
                    in_=ctx["Wo"][
                        c * 128 : (c + 1) * 128, half * 1024 : (half + 1) * 1024
                    ],
                )
            for r in range(RT):
                xres = xre.tile([128, 1024], BF16, tag="xre")
                nc.sync.dma_start(
                    out=xres,
                    in_=x_flat[
                        r * 128 : (r + 1) * 128, half * 1024 : (half + 1) * 1024
                    ],
                )
                for dc in range(2):
                    cols = slice(
                        half * 1024 + dc * 512, half * 1024 + (dc + 1) * 512
                    )
                    ps_o = ps.tile([128, 512], F32, tag="ps")
                    for c in range(DT):
                        nc.tensor.matmul(
                            ps_o,
                            ctxT[c][:, r * 128 : (r + 1) * 128],
                            wo_res[c][:, dc * 512 : (dc + 1) * 512],
                            start=(c == 0),
                            stop=(c == DT - 1),
                        )
                    nc.vector.tensor_add(
                        out=x2[r][:, cols],
                        in0=ps_o,
                        in1=xres[:, dc * 512 : (dc + 1) * 512],
                    )
                    if bo_b is not None:
                        nc.vector.tensor_add(
                            out=x2[r][:, cols], in0=x2[r][:, cols], in1=bo_b[:, cols]
                        )


def _phase_c(ctx, g2_id):
    nc = ctx["nc"]
    g2_b = be2_b = None
    if not g2_id:
        g2_b = _bcast_vec(ctx, ctx["g2"])
        be2_b = _bcast_vec(ctx, ctx["be2"])
    h2T = [
        ctx["feat2"].tile([128, NT], BF16, tag="feat2", name=f"h2T{c}")
        for c in range(DT)
    ]
    for r in range(RT):
        h2_t = ctx["stage2"].tile([128, D], BF16, tag="stage2")
        _layernorm_rows(ctx, ctx["x2"][r], h2_t, g2_b, be2_b)
        _transpose_rows(ctx, h2_t, h2T, r)
    return h2T


def _ffn_chunk(ctx, fc, h2T, b2_b, fch):
    nc, ps = ctx["nc"], ctx["ps"]
    x2, out_flat = ctx["x2"], ctx["out_flat"]
    b1_sb = ctx["b1_sb"]
    fct = FT // fch            # f-tiles per chunk
    fw = fct * 128             # chunk width in features
    # w1 tiles: [128, 2*fw] covering a d-pair, full chunk width
    w1_t = [
        ctx["w1p"].tile([128, 2 * fw], BF16, tag="w1", name=f"w1_{fc}_{g}")
        for g in range(8)
    ]
    for g in range(8):
        nc.sync.dma_start(
            out=w1_t[g].rearrange("p (t c) -> p t c", t=2),
            in_=ctx["W1"][
                g * 256 : (g + 1) * 256, fc * fw : (fc + 1) * fw
            ].rearrange("(t p) c -> p t c", p=128),
        )
    w2_t = [
        ctx["w2p"].tile([128, 2048], BF16, tag="w2", name=f"w2_{fc}_{i}")
        for i in range(fct)
    ]
    for i in range(fct):
        nc.sync.dma_start(
            out=w2_t[i],
            in_=ctx["W2"][(fc * fct + i) * 128 : (fc * fct + i + 1) * 128, :],
        )
    up_t = [
        ctx["upp"].tile([128, NT], BF16, tag="up", name=f"up{fc}_{i}")
        for i in range(fct)
    ]
    # up: for each f-tile, 2 chains (NT halves) over d
    for ft in range(fct):
        acc = [
            ps.tile([128, 512], F32, tag="ps", name=f"uacc{fc}_{ft}_{i}")
            for i in range(2)
        ]
        for d in range(DT):
            wd = w1_t[d // 2][:, (d % 2) * fw : (d % 2) * fw + fw]
            for b2i in range(2):
                nc.tensor.matmul(
                    acc[b2i],
                    wd[:, ft * 128 : (ft + 1) * 128],
                    h2T[d][:, b2i * 512 : (b2i + 1) * 512],
                    start=(d == 0),
                    stop=(d == DT - 1),
                )
        ftg = fc * fct + ft
        for b2i in range(2):
            nc.scalar.activation(
                out=up_t[ft][:, b2i * 512 : (b2i + 1) * 512],
                in_=acc[b2i],
                func=AF.Relu,
                bias=b1_sb[:, ftg : ftg + 1],
                scale=1.0,
            )
    # down: 32 chains (r, dc) over the chunk's f-tiles
    last = fc == fch - 1
    for r in range(RT):
        o_t = (
            ctx["outp"].tile([128, D], BF16, tag="outp", name=f"o_t{r}")
            if last
            else None
        )
        for dc in range(4):
            accd = ps.tile([128, 512], F32, tag="ps")
            for i in range(fct):
                nc.tensor.matmul(
                    accd,
                    up_t[i][:, r * 128 : (r + 1) * 128],
                    w2_t[i][:, dc * 512 : (dc + 1) * 512],
                    start=(i == 0),
                    stop=(i == fct - 1),
                )
            cols = slice(dc * 512, (dc + 1) * 512)
            if not last:
                nc.vector.tensor_add(
                    out=x2[r][:, cols], in0=x2[r][:, cols], in1=accd
                )
            elif b2_b is None:
                nc.vector.tensor_add(out=o_t[:, cols], in0=x2[r][:, cols], in1=accd)
            else:
                nc.vector.tensor_add(
                    out=x2[r][:, cols], in0=x2[r][:, cols], in1=accd
                )
                nc.vector.tensor_add(
                    out=o_t[:, cols], in0=x2[r][:, cols], in1=b2_b[:, cols]
                )
        if last:
            nc.sync.dma_start(out=out_flat[r * 128 : (r + 1) * 128, :], in_=o_t)


def _phase_cde(ctx, g2_id, b2_zero, fast):
    tc = ctx["tc"]
    fch = 2 * FCH
    fct = FT // fch
    with (
        tc.tile_pool(name="feat2", bufs=DT)
    ) as feat2, (
        tc.tile_pool(name="stage2", bufs=2 if fast else 1)
    ) as stage2, (
        tc.tile_pool(name="lnt2", bufs=1)
    ) as lnt2:
        ctx.update(feat2=feat2, stage2=stage2, lnt=lnt2)
        h2T = _phase_c(ctx, g2_id)
        b2_b = None if b2_zero else _bcast_vec(ctx, ctx["b2"])
        with (
            tc.tile_pool(name="w1p", bufs=8)
        ) as w1p, (
            tc.tile_pool(name="w2p", bufs=fct)
        ) as w2p, (
            tc.tile_pool(name="upp", bufs=fct + (2 if fast else 0))
        ) as upp, (
            tc.tile_pool(name="outp", bufs=2 if fast else 1)
        ) as outp:
            ctx.update(w1p=w1p, w2p=w2p, upp=upp, outp=outp)
            for fc in range(fch):
                _ffn_chunk(ctx, fc, h2T, b2_b, fch)


def build_module(g1_id=False, g2_id=False, bo_zero=False, b2_zero=False):
    nc = bass.Bass()

    dram = {}
    dram["x"] = nc.dram_tensor("x", [BPC, T, D], BF16, kind="ExternalInput")
    dram["Wq"] = nc.dram_tensor("Wq", [H, D, HD], BF16, kind="ExternalInput")
    dram["Wk"] = nc.dram_tensor("Wk", [H, D, HD], BF16, kind="ExternalInput")
    dram["Wv"] = nc.dram_tensor("Wv", [H, D, HD], BF16, kind="ExternalInput")
    dram["Wo"] = nc.dram_tensor("Wo", [H * HD, D], BF16, kind="ExternalInput")
    dram["bo"] = nc.dram_tensor("bo", [D], F32, kind="ExternalInput")
    dram["W1"] = nc.dram_tensor("W1", [D, F], BF16, kind="ExternalInput")
    dram["b1"] = nc.dram_tensor("b1", [F], F32, kind="ExternalInput")
    dram["W2"] = nc.dram_tensor("W2", [F, D], BF16, kind="ExternalInput")
    dram["b2"] = nc.dram_tensor("b2", [D], F32, kind="ExternalInput")
    dram["g1"] = nc.dram_tensor("g1", [D], F32, kind="ExternalInput")
    dram["be1"] = nc.dram_tensor("be1", [D], F32, kind="ExternalInput")
    dram["g2"] = nc.dram_tensor("g2", [D], F32, kind="ExternalInput")
    dram["be2"] = nc.dram_tensor("be2", [D], F32, kind="ExternalInput")
    out = nc.dram_tensor("out", [BPC, T, D], BF16, kind="ExternalOutput")

    ctx = dict(dram)
    ctx["nc"] = nc
    ctx["x_flat"] = dram["x"][:, :, :].flatten_outer_dims()
    ctx["out_flat"] = out[:, :, :].flatten_outer_dims()

    with tile.TileContext(nc) as tc, (
        tc.tile_pool(name="cst", bufs=1)
    ) as cst, (
        tc.tile_pool(name="small", bufs=6)
    ) as small, (
        tc.tile_pool(name="bcast", bufs=2)
    ) as bcast, (
        tc.tile_pool(name="ps", bufs=8, space="PSUM")
    ) as ps:
        ctx.update(tc=tc, cst=cst, small=small, bcast=bcast, ps=ps)

        ident = cst.tile([128, 128], BF16)
        make_identity(nc, ident)
        mask = cst.tile([128, 128], F32)
        nc.gpsimd.memset(mask, 0.0)
        nc.gpsimd.affine_select(
            out=mask, in_=mask, compare_op=ALU.is_ge, fill=NEG,
            base=0, pattern=[[-1, 128]], channel_multiplier=1,
        )
        eps_sb = cst.tile([128, 1], F32)
        nc.vector.memset(eps_sb, EPS)
        b1_sb = cst.tile([128, FT], F32)
        nc.sync.dma_start(out=b1_sb, in_=dram["b1"].rearrange("(f p) -> p f", p=128))
        ctx.update(ident=ident, mask=mask, eps_sb=eps_sb, b1_sb=b1_sb)

        with tc.tile_pool(name="ctxp", bufs=DT) as ctxp:
            ctx["ctxT"] = [
                ctxp.tile([128, NT], BF16, tag="ctx", name=f"ctxT{c}")
                for c in range(DT)
            ]
            _phase_ab(ctx, g1_id)

            with tc.tile_pool(name="x2p", bufs=RT) as x2p:
                ctx["x2"] = [
                    x2p.tile([128, D], F32, tag="x2p", name=f"x2_{c}")
                    for c in range(RT)
                ]
                _phase_wo(ctx, bo_zero)
                _phase_cde(
                    ctx, g2_id, b2_zero,
                    g1_id and g2_id and bo_zero and b2_zero,
                )

    _split_sync_waits(nc)
    return nc


def _f32(a):
    return np.ascontiguousarray(np.asarray(a), dtype=np.float32)


def _flags_from_inputs(inputs):
    return (
        bool(np.all(_f32(inputs["g1"]) == 1.0) and np.all(_f32(inputs["be1"]) == 0.0)),
        bool(np.all(_f32(inputs["g2"]) == 1.0) and np.all(_f32(inputs["be2"]) == 0.0)),
        bool(np.all(_f32(inputs["bo"]) == 0.0)),
        bool(np.all(_f32(inputs["b2"]) == 0.0)),
    )


def _to_bf16(a):
    import ml_dtypes

    return np.ascontiguousarray(np.asarray(a).astype(ml_dtypes.bfloat16))


class _AxonRunner:
    """Compile-once PJRT runner for the axon (remote NeuronCore) backend."""

    def __init__(self, flags):
        import jax
        from jax.sharding import Mesh, NamedSharding, PartitionSpec
        from concourse import bass2jax

        try:
            jax.config.update("jax_compilation_cache_dir", "/tmp/jax_comp_cache")
            jax.config.update("jax_persistent_cache_min_compile_time_secs", 10)
        except Exception:
            pass

        self.jax = jax

        bass2jax.install_neuronx_cc_hook()
        nc = self.nc = build_module(*flags)
        assert nc.dbg_addr is None, "debug build not supported in cached runner"
        partition_name = (
            nc.partition_id_tensor.name if nc.partition_id_tensor else None
        )

        in_names = []
        out_names = []
        out_avals = []
        for alloc in nc.m.functions[0].allocations:
            if not isinstance(alloc, mybir.MemoryLocationSet):
                continue
            name = alloc.memorylocations[0].name
            if alloc.kind == "ExternalInput":
                if name != partition_name:
                    in_names.append(name)
            elif alloc.kind == "ExternalOutput":
                out_names.append(name)
                out_avals.append(
                    jax.core.ShapedArray(
                        tuple(alloc.tensor_shape), mybir.dt.np(alloc.dtype)
                    )
                )
        self.in_names = in_names
        self.out_names = out_names
        n_params = len(in_names)
        n_outs = len(out_names)
        all_in_names = tuple(in_names + out_names)
        if partition_name is not None:
            all_in_names = all_in_names + (partition_name,)

        devices = jax.devices()[:N_CORES]
        assert len(devices) == N_CORES
        mesh = Mesh(np.asarray(devices), ("core",))
        self.mesh = mesh
        self.sh_core = NamedSharding(mesh, PartitionSpec("core"))
        self.sh_rep = NamedSharding(mesh, PartitionSpec(None))

        # x and the out-params are batch-sharded; weights are replicated.
        def spec_for(name):
            return PartitionSpec("core") if name in ("x", "out") else PartitionSpec(None)

        in_specs = tuple(spec_for(n) for n in in_names + out_names)
        out_specs = tuple(PartitionSpec("core") for _ in out_names)

        def _body(*args):
            operands = list(args)
            if partition_name is not None:
                operands.append(bass2jax.partition_id_tensor())
            outs = bass2jax._bass_exec_p.bind(
                *operands,
                out_avals=tuple(out_avals),
                in_names=all_in_names,
                out_names=tuple(out_names),
                lowering_input_output_aliases=(),
                sim_require_finite=True,
                sim_require_nnan=True,
                nc=nc,
            )
            return tuple(outs)

        from jax.experimental.shard_map import shard_map as _shard_map

        smap = _shard_map(
            _body, mesh=mesh, in_specs=in_specs, out_specs=out_specs,
            check_rep=False,
        )
        donate = tuple(range(n_params, n_params + n_outs))
        self.fn = jax.jit(smap, donate_argnums=donate, keep_unused=True)

        self._weight_key = None
        self._weight_dev = None
        self._donate_out = None

    def _weights_to_device(self, inputs):
        key = tuple(
            (id(inputs[k]), np.asarray(inputs[k]).shape) for k in WEIGHT_NAMES
        )
        if key == self._weight_key:
            return
        put = []
        for k in WEIGHT_NAMES:
            a = _to_bf16(inputs[k]) if k in BF16_WEIGHTS else _f32(inputs[k])
            put.append(self.jax.device_put(a, self.sh_rep))
        for a in put:
            a.block_until_ready()
        self._weight_dev = dict(zip(WEIGHT_NAMES, put))
        self._weight_key = key

    def _put_sharded_bf16(self, arr_f32):
        """Cast to bf16 and upload, one concurrent cast+put per device."""
        import concurrent.futures as cf

        import ml_dtypes

        jax = self.jax
        devs = list(self.mesh.devices.flat)
        n = len(devs)
        per = arr_f32.shape[0] // n

        def put(i):
            piece = np.ascontiguousarray(
                arr_f32[i * per : (i + 1) * per].astype(ml_dtypes.bfloat16)
            )
            return jax.device_put(piece, devs[i])

        with cf.ThreadPoolExecutor(n) as ex:
            bufs = list(ex.map(put, range(n)))
        return jax.make_array_from_single_device_arrays(
            (N_CORES * BPC, T, D), self.sh_core, bufs
        )

    def _get_sharded_f32(self, out):
        """Fetch shards concurrently, converting bf16->f32 in the fetch
        threads directly into a preallocated full-shape f32 array."""
        import concurrent.futures as cf

        shards = sorted(out.addressable_shards, key=lambda s: s.index)
        for s in shards:
            try:
                s.data.copy_to_host_async()
            except Exception:
                pass
        res = np.empty((B, T, D), np.float32)

        def fetch(i):
            lo = i * BPC
            buf = np.asarray(shards[i].data)  # bf16, host
            res[lo : lo + BPC] = buf.astype(np.float32)

        with cf.ThreadPoolExecutor(len(shards)) as ex:
            list(ex.map(fetch, range(len(shards))))
        return res

    def __call__(self, inputs):
        import time

        import ml_dtypes

        jax = self.jax
        prof = bool(os.environ.get("KERNEL_PROFILE"))
        threaded = os.environ.get("KERNEL_THREADED", "1") != "0"
        t0 = time.time()
        self._weights_to_device(inputs)
        t1 = time.time()
        x_np = _f32(inputs["x"])
        t2 = time.time()
        if threaded:
            xd = self._put_sharded_bf16(x_np)
        else:
            xd = jax.device_put(_to_bf16(x_np), self.sh_core)
        xd.block_until_ready()
        t3 = time.time()
        if self._donate_out is None:
            zeros = np.zeros((N_CORES * BPC, T, D), ml_dtypes.bfloat16)
            self._donate_out = jax.device_put(zeros, self.sh_core)
        args = []
        for n in self.in_names:
            if n == "x":
                args.append(xd)
            else:
                args.append(self._weight_dev[n])
        args.append(self._donate_out)
        # the donated buffer is consumed even if the call fails; drop the
        # reference first so a failure doesn't leave a dead array cached
        self._donate_out = None
        (out,) = self.fn(*args)
        out.block_until_ready()
        t4 = time.time()
        if threaded:
            final = self._get_sharded_f32(out)
        else:
            final = np.asarray(out).reshape(B, T, D).astype(np.float32)
        t5 = time.time()
        # out is fully written by the kernel; reuse it as the next donated
        # out-param so no fresh buffer ever needs uploading.
        self._donate_out = out
        if prof:
            print(
                f"[kprof] weights={t1-t0:.3f} cast={t2-t1:.3f} "
                f"upload={t3-t2:.3f} exec={t4-t3:.3f} fetch+conv={t5-t4:.3f} "
                f"total={t5-t0:.3f}",
                flush=True,
            )
        return final


_RUNNERS = {}
_NC_CACHE = {}


def _kernel_axon(inputs):
    flags = _flags_from_inputs(inputs)
    if flags not in _RUNNERS:
        _RUNNERS[flags] = _AxonRunner(flags)
    return _RUNNERS[flags](inputs)


_NATIVE_W_CACHE = {}


def _kernel_native(inputs):
    flags = _flags_from_inputs(inputs)
    if flags not in _NC_CACHE:
        _NC_CACHE[flags] = build_module(*flags)
    nc = _NC_CACHE[flags]

    xs = _to_bf16(inputs["x"])
    wkey = tuple(id(inputs[k]) for k in WEIGHT_NAMES)
    shared = _NATIVE_W_CACHE.get(wkey)
    if shared is None:
        shared = {
            k: (_to_bf16(inputs[k]) if k in BF16_WEIGHTS else _f32(inputs[k]))
            for k in WEIGHT_NAMES
        }
        _NATIVE_W_CACHE.clear()
        _NATIVE_W_CACHE[wkey] = shared
    in_maps = []
    for i in range(N_CORES):
        m = dict(shared)
        m["x"] = xs[i * BPC : (i + 1) * BPC]
        in_maps.append(m)

    res = run_bass_kernel_spmd(nc, in_maps, core_ids=list(range(N_CORES)))
    return np.concatenate(
        [res.results[i]["out"].astype(np.float32) for i in range(N_CORES)], axis=0
    )


def kernel(**inputs):
    if axon_active():
        try:
            return _kernel_axon(inputs)
        except Exception:
            import traceback

            traceback.print_exc()
            if os.environ.get("KERNEL_NO_FALLBACK"):
                raise
            return _kernel_native(inputs)
    return _kernel_native(inputs)


# revision 5
# speedup vs baseline: 2.3797x; 2.3797x over previous
"""Trainium2 Bass kernel for a pre-LN causal transformer block (v3).

Sharding: data-parallel over batch. B=16 split across 8 NeuronCores
(2 sequences per core); weights replicated. No collectives needed.

v3 device changes vs the baseline kernel:
  - all matmuls in bf16 (fp32 PSUM accumulation): same PE rate as f32r
    but half the SBUF/DMA footprint;
  - weights are uploaded pre-cast to bf16, x/out cross the host link in
    bf16 (tolerance is 2e-2; measured end-to-end error ~2e-3);
  - FFN up/down fused per F-chunk with the up activations kept in SBUF
    (the old kernel spilled 33MB of upT to DRAM and read it back);
  - ctxT kept in SBUF (old kernel round-tripped it through DRAM);
  - far fewer, larger weight DMAs with deep prefetch pools (the old
    kernel's 4-matmul bursts stalled ~2-3us on every 128KB weight DMA);
  - PE transposes packed 4-per-PSUM-bank, drains split ACT/DVE.

Host path: module compiled once per process, jitted PJRT callable cached,
weights device_put replicated once (keyed by caller array identity);
steady-state calls move only x (bf16) in and out (bf16) back.
"""

import os
import sys

import numpy as np

for _p in ("/opt/trn_rl_repo", "/root/.axon_site/_ro/trn_rl_repo"):
    if os.path.isdir(_p) and _p not in sys.path:
        sys.path.append(_p)

import concourse.bass as bass
import concourse.tile as tile
from concourse import mybir
from concourse._compat import axon_active
from concourse.bass_utils import run_bass_kernel_spmd
from concourse.masks import make_identity

F32 = mybir.dt.float32
BF16 = mybir.dt.bfloat16
AX = mybir.AxisListType.X
AF = mybir.ActivationFunctionType
ALU = mybir.AluOpType

B, T, D, H = 16, 512, 2048, 4
HD = D // H            # 512
F = 4 * D              # 8192
N_CORES = 8
BPC = B // N_CORES     # 2 sequences per core
NT = BPC * T           # 1024 tokens per core
EPS = 1e-5
NEG = -1.0e30

DT = D // 128          # 16 d-tiles
RT = NT // 128         # 8 token-row tiles
KT = HD // 128         # 4 head-dim tiles
ST = T // 128          # 4 seq tiles per sequence
FT = F // 128          # 64 ffn tiles
FCH = 8                # ffn f-chunks
FCT = FT // FCH        # 8 f-tiles per chunk

_MAXW = 1

WEIGHT_NAMES = (
    "Wq", "Wk", "Wv", "Wo", "bo", "W1", "b1", "W2", "b2",
    "g1", "be1", "g2", "be2",
)
BF16_WEIGHTS = {"Wq", "Wk", "Wv", "Wo", "W1", "W2"}


def _split_sync_waits(nc):
    """Hoist excess semaphore waits into standalone same-engine drains."""
    for f in nc.m.functions:
        for blk in f.blocks:
            newl = []
            for inst in blk.instructions:
                si = inst.sync_info
                if si is not None and si.on_wait and len(si.on_wait) > _MAXW:
                    waits = list(si.on_wait)
                    extra, keep = waits[:-_MAXW], waits[-_MAXW:]
                    for i in range(0, len(extra), _MAXW):
                        d = mybir.InstDrain(
                            name=f"{inst.name}-wsplit{i}", ins=[], outs=[]
                        )
                        d.engine = inst.engine
                        d.sync_info = mybir.SyncInfo(
                            on_wait=extra[i : i + _MAXW], on_update=[]
                        )
                        newl.append(d)
                    inst.sync_info = mybir.SyncInfo(
                        on_wait=keep, on_update=list(si.on_update)
                    )
                newl.append(inst)
            blk.instructions = newl


def _layernorm_rows(ctx, src_ap, dst_tile, g_b=None, be_b=None):
    """dst(bf16) = LN(src) * g + be for one [128, D] row tile."""
    nc, small = ctx["nc"], ctx["small"]
    eps_sb = ctx["eps_sb"]
    stats = small.tile([128, 4, 6], F32, tag="stats")
    xr = src_ap.rearrange("p (c f) -> p c f", c=4)
    for c in range(4):
        nc.vector.bn_stats(out=stats[:, c, :], in_=xr[:, c, :])
    mv = small.tile([128, 2], F32, tag="mv")
    nc.vector.bn_aggr(out=mv, in_=stats)
    rstd = small.tile([128, 1], F32, tag="rstd")
    nc.scalar.activation(
        out=rstd, in_=mv[:, 1:2], func=AF.Sqrt, bias=eps_sb, scale=1.0
    )
    nc.vector.reciprocal(out=rstd, in_=rstd)
    if g_b is None and be_b is None:
        nc.vector.tensor_scalar(
            out=dst_tile, in0=src_ap, scalar1=mv[:, 0:1], scalar2=rstd,
            op0=ALU.subtract, op1=ALU.mult,
        )
        return
    tmp = ctx["lnt"].tile([128, D], F32, tag="lntmp")
    nc.vector.tensor_scalar(
        out=tmp, in0=src_ap, scalar1=mv[:, 0:1], scalar2=rstd,
        op0=ALU.subtract, op1=ALU.mult,
    )
    if g_b is not None:
        nc.vector.tensor_mul(out=tmp, in0=tmp, in1=g_b)
    if be_b is not None:
        nc.vector.tensor_add(out=dst_tile, in0=tmp, in1=be_b)
    else:
        nc.vector.tensor_copy(out=dst_tile, in_=tmp)


def _transpose_rows(ctx, src_tile, dst_tiles, r):
    """PE-transpose [128, n_c*128] bf16 row tile r into dst_tiles[c][:, r-slice].

    Transposes are packed 4 per PSUM bank; drains alternate ACT/DVE."""
    nc, ps, ident = ctx["nc"], ctx["ps"], ctx["ident"]
    n_c = src_tile.shape[-1] // 128
    for g in range((n_c + 3) // 4):
        cs = list(range(g * 4, min(g * 4 + 4, n_c)))
        pt = ps.tile([128, 512], BF16, tag="ps")
        for i, c in enumerate(cs):
            nc.tensor.transpose(
                pt[:, i * 128 : (i + 1) * 128],
                src_tile[:, c * 128 : (c + 1) * 128],
                ident,
            )
        for i, c in enumerate(cs):
            dst = dst_tiles[c][:, r * 128 : (r + 1) * 128]
            src = pt[:, i * 128 : (i + 1) * 128]
            if c % 2 == 0:
                nc.scalar.copy(out=dst, in_=src)
            else:
                nc.vector.tensor_copy(out=dst, in_=src)


def _bcast_vec(ctx, dram_vec):
    nc = ctx["nc"]
    t = ctx["bcast"].tile([128, D], F32, tag="bc")
    nc.sync.dma_start(out=t, in_=dram_vec[None, :].to_broadcast((128, D)))
    return t


def _phase_a(ctx, g1_id):
    """LN1(x) + PE-transpose -> hT [D, NT] bf16. Returns hT tiles."""
    nc, tc = ctx["nc"], ctx["tc"]
    g1_b = be1_b = None
    if not g1_id:
        g1_b = _bcast_vec(ctx, ctx["g1"])
        be1_b = _bcast_vec(ctx, ctx["be1"])
    feat = ctx["feat"]
    stage = ctx["stage"]
    hT = [feat.tile([128, NT], BF16, tag="feat", name=f"hT{c}") for c in range(DT)]
    x_flat = ctx["x_flat"]
    for r in range(RT):
        x_t = stage.tile([128, D], BF16, tag="xstage")
        nc.sync.dma_start(out=x_t, in_=x_flat[r * 128 : (r + 1) * 128, :])
        h_t = stage.tile([128, D], BF16, tag="hstage")
        _layernorm_rows(ctx, x_t, h_t, g1_b, be1_b)
        _transpose_rows(ctx, h_t, hT, r)
    return hT


def _attn_head(ctx, h, hT):
    """One attention head: QKV projections, causal softmax, PV -> ctxT."""
    nc, ps = ctx["nc"], ctx["ps"]
    ident, mask, small = ctx["ident"], ctx["mask"], ctx["small"]
    ctxT = ctx["ctxT"]
    scale = float(HD) ** -0.5

    wq_t = [ctx["wqp"].tile([128, 1024], BF16, tag="wq", name=f"wq{h}_{g}")
            for g in range(8)]
    wk_t = [ctx["wkp"].tile([128, 1024], BF16, tag="wk", name=f"wk{h}_{g}")
            for g in range(8)]
    wv_t = [ctx["wvp"].tile([128, 1024], BF16, tag="wv", name=f"wv{h}_{g}")
            for g in range(8)]
    for g in range(8):
        for wt, wsrc in ((wq_t, ctx["Wq"]), (wk_t, ctx["Wk"]), (wv_t, ctx["Wv"])):
            nc.sync.dma_start(
                out=wt[g].rearrange("p (t c) -> p t c", t=2),
                in_=wsrc[h, g * 256 : (g + 1) * 256, :].rearrange(
                    "(t p) c -> p t c", p=128
                ),
            )

    qT = [ctx["qp"].tile([128, NT], BF16, tag="qp", name=f"qT{h}_{c}")
          for c in range(KT)]
    kT = [ctx["kp"].tile([128, NT], BF16, tag="kp", name=f"kT{h}_{c}")
          for c in range(KT)]
    v_sb = [ctx["vp"].tile([128, HD], BF16, tag="vp", name=f"v{h}_{c}")
            for c in range(BPC * ST)]

    # Q and K: out[hd, t] = W.T @ h ; 4 chains (kth2, b) per half
    for proj, w_t, dsts in (("q", wq_t, qT), ("k", wk_t, kT)):
        for half in range(2):
            acc = [
                ps.tile([128, 512], F32, tag="ps", name=f"{proj}acc{h}_{half}_{i}")
                for i in range(4)
            ]
            for d in range(DT):
                wd = w_t[d // 2][:, (d % 2) * 512 : (d % 2) * 512 + 512]
                for kth2 in range(2):
                    kth = half * 2 + kth2
                    for b in range(BPC):
                        nc.tensor.matmul(
                            acc[kth2 * 2 + b],
                            wd[:, kth * 128 : (kth + 1) * 128],
                            hT[d][:, b * T : (b + 1) * T],
                            start=(d == 0),
                            stop=(d == DT - 1),
                        )
            for kth2 in range(2):
                kth = half * 2 + kth2
                for b in range(BPC):
                    dst = dsts[kth][:, b * T : (b + 1) * T]
                    if proj == "q":
                        nc.scalar.mul(out=dst, in_=acc[kth2 * 2 + b], mul=scale)
                    else:
                        nc.vector.tensor_copy(out=dst, in_=acc[kth2 * 2 + b])

    # V: out[t, hd] = h @ Wv ; 4 chains (st) per b
    for b in range(BPC):
        acc = [
            ps.tile([128, 512], F32, tag="ps", name=f"vacc{h}_{b}_{i}")
            for i in range(ST)
        ]
        for d in range(DT):
            wd = wv_t[d // 2][:, (d % 2) * 512 : (d % 2) * 512 + 512]
            for st in range(ST):
                nc.tensor.matmul(
                    acc[st],
                    hT[d][:, b * T + st * 128 : b * T + (st + 1) * 128],
                    wd,
                    start=(d == 0),
                    stop=(d == DT - 1),
                )
        for st in range(ST):
            nc.vector.tensor_copy(out=v_sb[b * ST + st], in_=acc[st])

    # causal softmax + PV per sequence
    for b in range(BPC):
        attnT = [
            ctx["attntp"].tile([128, T], BF16, tag="attntp", name=f"attnT{h}_{b}_{c}")
            for c in range(ST)
        ]
        for tt in range(ST):
            ns = 128 * (tt + 1)
            ps_s = ps.tile([128, 512], F32, tag="ps")
            for kt in range(KT):
                nc.tensor.matmul(
                    ps_s[:, :ns],
                    qT[kt][:, b * T + tt * 128 : b * T + (tt + 1) * 128],
                    kT[kt][:, b * T : b * T + ns],
                    start=(kt == 0),
                    stop=(kt == KT - 1),
                )
            nc.vector.tensor_add(
                out=ps_s[:, tt * 128 : ns], in0=ps_s[:, tt * 128 : ns], in1=mask
            )
            negmax = small.tile([128, 1], F32, tag="negmax")
            nc.vector.reduce_max(out=negmax, in_=ps_s[:, :ns], axis=AX, negate=True)
            attn_t = ctx["attnp"].tile([128, T], BF16, tag="attnp")
            rowsum = small.tile([128, 1], F32, tag="rowsum")
            nc.scalar.activation(
                out=attn_t[:, :ns], in_=ps_s[:, :ns], func=AF.Exp,
                bias=negmax, scale=1.0, accum_out=rowsum,
            )
            rinv = small.tile([128, 1], F32, tag="rinv")
            nc.vector.reciprocal(out=rinv, in_=rowsum)
            nc.vector.tensor_scalar_mul(
                out=attn_t[:, :ns], in0=attn_t[:, :ns], scalar1=rinv
            )
            pt = ps.tile([128, 512], BF16, tag="ps")
            for st in range(tt + 1):
                nc.tensor.transpose(
                    pt[:, st * 128 : (st + 1) * 128],
                    attn_t[:, st * 128 : (st + 1) * 128],
                    ident,
                )
            for st in range(tt + 1):
                dst = attnT[st][:, tt * 128 : (tt + 1) * 128]
                src = pt[:, st * 128 : (st + 1) * 128]
                if st % 2 == 0:
                    nc.scalar.copy(out=dst, in_=src)
                else:
                    nc.vector.tensor_copy(out=dst, in_=src)
        for ht in range(KT):
            ps_c = ps.tile([128, 512], F32, tag="ps")
            for st in range(ST):
                # causal: s-tile st only feeds outputs t >= st*128
                nc.tensor.matmul(
                    ps_c[:, st * 128 : T],
                    v_sb[b * ST + st][:, ht * 128 : (ht + 1) * 128],
                    attnT[st][:, st * 128 : T],
                    start=(st == 0),
                    stop=(st == ST - 1),
                    skip_group_check=True,
                )
            nc.vector.tensor_copy(
                out=ctxT[h * KT + ht][:, b * T : (b + 1) * T], in_=ps_c
            )


def _phase_ab(ctx, g1_id):
    tc = ctx["tc"]
    with (
        tc.tile_pool(name="feat", bufs=DT)
    ) as feat, (
        tc.tile_pool(name="stage", bufs=2)
    ) as stage, (
        tc.tile_pool(name="lnt", bufs=1)
    ) as lnt, (
        tc.tile_pool(name="wqp", bufs=8)
    ) as wqp, (
        tc.tile_pool(name="wkp", bufs=8)
    ) as wkp, (
        tc.tile_pool(name="wvp", bufs=8)
    ) as wvp, (
        tc.tile_pool(name="qp", bufs=6)
    ) as qp, (
        tc.tile_pool(name="kp", bufs=6)
    ) as kp, (
        tc.tile_pool(name="vp", bufs=12)
    ) as vp, (
        tc.tile_pool(name="attnp", bufs=3)
    ) as attnp, (
        tc.tile_pool(name="attntp", bufs=8)
    ) as attntp:
        ctx.update(
            feat=feat, stage=stage, lnt=lnt, wqp=wqp, wkp=wkp, wvp=wvp,
            qp=qp, kp=kp, vp=vp, attnp=attnp, attntp=attntp,
        )
        hT = _phase_a(ctx, g1_id)
        for h in range(H):
            _attn_head(ctx, h, hT)


def _phase_wo(ctx, bo_zero):
    """x2 = x + ctxT.T @ Wo (+bo), f32, SBUF-resident."""
    nc, tc, ps = ctx["nc"], ctx["tc"], ctx["ps"]
    ctxT, x_flat = ctx["ctxT"], ctx["x_flat"]
    bo_b = None if bo_zero else _bcast_vec(ctx, ctx["bo"])
    x2 = ctx["x2"]
    with (
        tc.tile_pool(name="wop", bufs=24)
    ) as wop, (
        tc.tile_pool(name="xre", bufs=3)
    ) as xre:
        for half in range(2):
            wo_res = [
                wop.tile([128, 1024], BF16, tag="wop", name=f"wo{half}_{c}")
                for c in range(DT)
            ]
            for c in range(DT):
                nc.sync.dma_start(
                    out=wo_res[c],
                    in_=ctx["Wo"][
                        c * 128 : (c + 1) * 128, half * 1024 : (half + 1) * 1024
                    ],
                )
            for r in range(RT):
                xres = xre.tile([128, 1024], BF16, tag="xre")
                nc.sync.dma_start(
                    out=xres,
                    in_=x_flat[
                        r * 128 : (r + 1) * 128, half * 1024 : (half + 1) * 1024
                    ],
                )
                for dc in range(2):
                    cols = slice(
                        half * 1024 + dc * 512, half * 1024 + (dc + 1) * 512
                    )
                    ps_o = ps.tile([128, 512], F32, tag="ps")
                    for c in range(DT):
                        nc.tensor.matmul(
                            ps_o,
                            ctxT[c][:, r * 128 : (r + 1) * 128],
                            wo_res[c][:, dc * 512 : (dc + 1) * 512],
                            start=(c == 0),
                            stop=(c == DT - 1),
                        )
                    nc.vector.tensor_add(
                        out=x2[r][:, cols],
                        in0=ps_o,
                        in1=xres[:, dc * 512 : (dc + 1) * 512],
                    )
                    if bo_b is not None:
                        nc.vector.tensor_add(
                            out=x2[r][:, cols], in0=x2[r][:, cols], in1=bo_b[:, cols]
                        )


def _phase_c(ctx, g2_id):
    nc = ctx["nc"]
    g2_b = be2_b = None
    if not g2_id:
        g2_b = _bcast_vec(ctx, ctx["g2"])
        be2_b = _bcast_vec(ctx, ctx["be2"])
    h2T = [
        ctx["feat2"].tile([128, NT], BF16, tag="feat2", name=f"h2T{c}")
        for c in range(DT)
    ]
    for r in range(RT):
        h2_t = ctx["stage2"].tile([128, D], BF16, tag="stage2")
        _layernorm_rows(ctx, ctx["x2"][r], h2_t, g2_b, be2_b)
        _transpose_rows(ctx, h2_t, h2T, r)
    return h2T


def _ffn_chunk(ctx, fc, h2T, b2_b, fch):
    nc, ps = ctx["nc"], ctx["ps"]
    x2, out_flat = ctx["x2"], ctx["out_flat"]
    b1_sb = ctx["b1_sb"]
    fct = FT // fch            # f-tiles per chunk
    fw = fct * 128             # chunk width in features
    # w1 tiles: [128, 2*fw] covering a d-pair, full chunk width
    w1_t = [
        ctx["w1p"].tile([128, 2 * fw], BF16, tag="w1", name=f"w1_{fc}_{g}")
        for g in range(8)
    ]
    for g in range(8):
        nc.sync.dma_start(
            out=w1_t[g].rearrange("p (t c) -> p t c", t=2),
            in_=ctx["W1"][
                g * 256 : (g + 1) * 256, fc * fw : (fc + 1) * fw
            ].rearrange("(t p) c -> p t c", p=128),
        )
    w2_t = [
        ctx["w2p"].tile([128, 2048], BF16, tag="w2", name=f"w2_{fc}_{i}")
        for i in range(fct)
    ]
    for i in range(fct):
        nc.sync.dma_start(
            out=w2_t[i],
            in_=ctx["W2"][(fc * fct + i) * 128 : (fc * fct + i + 1) * 128, :],
        )
    up_t = [
        ctx["upp"].tile([128, NT], BF16, tag="up", name=f"up{fc}_{i}")
        for i in range(fct)
    ]
    # up: for each f-tile, 2 chains (NT halves) over d
    for ft in range(fct):
        acc = [
            ps.tile([128, 512], F32, tag="ps", name=f"uacc{fc}_{ft}_{i}")
            for i in range(2)
        ]
        for d in range(DT):
            wd = w1_t[d // 2][:, (d % 2) * fw : (d % 2) * fw + fw]
            for b2i in range(2):
                nc.tensor.matmul(
                    acc[b2i],
                    wd[:, ft * 128 : (ft + 1) * 128],
                    h2T[d][:, b2i * 512 : (b2i + 1) * 512],
                    start=(d == 0),
                    stop=(d == DT - 1),
                )
        ftg = fc * fct + ft
        for b2i in range(2):
            nc.scalar.activation(
                out=up_t[ft][:, b2i * 512 : (b2i + 1) * 512],
                in_=acc[b2i],
                func=AF.Relu,
                bias=b1_sb[:, ftg : ftg + 1],
                scale=1.0,
            )
    # down: 32 chains (r, dc) over the chunk's f-tiles
    last = fc == fch - 1
    for r in range(RT):
        o_t = (
            ctx["outp"].tile([128, D], BF16, tag="outp", name=f"o_t{r}")
            if last
            else None
        )
        for dc in range(4):
            accd = ps.tile([128, 512], F32, tag="ps")
            for i in range(fct):
                nc.tensor.matmul(
                    accd,
                    up_t[i][:, r * 128 : (r + 1) * 128],
                    w2_t[i][:, dc * 512 : (dc + 1) * 512],
                    start=(i == 0),
                    stop=(i == fct - 1),
                )
            cols = slice(dc * 512, (dc + 1) * 512)
            if not last:
                nc.vector.tensor_add(
                    out=x2[r][:, cols], in0=x2[r][:, cols], in1=accd
                )
            elif b2_b is None:
                nc.vector.tensor_add(out=o_t[:, cols], in0=x2[r][:, cols], in1=accd)
            else:
                nc.vector.tensor_add(
                    out=x2[r][:, cols], in0=x2[r][:, cols], in1=accd
                )
                nc.vector.tensor_add(
                    out=o_t[:, cols], in0=x2[r][:, cols], in1=b2_b[:, cols]
                )
        if last:
            nc.sync.dma_start(out=out_flat[r * 128 : (r + 1) * 128, :], in_=o_t)


def _phase_cde(ctx, g2_id, b2_zero, fast):
    tc = ctx["tc"]
    fch = 2 * FCH
    fct = FT // fch
    with (
        tc.tile_pool(name="feat2", bufs=DT)
    ) as feat2, (
        tc.tile_pool(name="stage2", bufs=2 if fast else 1)
    ) as stage2, (
        tc.tile_pool(name="lnt2", bufs=1)
    ) as lnt2:
        ctx.update(feat2=feat2, stage2=stage2, lnt=lnt2)
        h2T = _phase_c(ctx, g2_id)
        b2_b = None if b2_zero else _bcast_vec(ctx, ctx["b2"])
        with (
            tc.tile_pool(name="w1p", bufs=8)
        ) as w1p, (
            tc.tile_pool(name="w2p", bufs=fct)
        ) as w2p, (
            tc.tile_pool(name="upp", bufs=fct + (2 if fast else 0))
        ) as upp, (
            tc.tile_pool(name="outp", bufs=2 if fast else 1)
        ) as outp:
            ctx.update(w1p=w1p, w2p=w2p, upp=upp, outp=outp)
            for fc in range(fch):
                _ffn_chunk(ctx, fc, h2T, b2_b, fch)


def build_module(g1_id=False, g2_id=False, bo_zero=False, b2_zero=False):
    nc = bass.Bass()

    dram = {}
    dram["x"] = nc.dram_tensor("x", [BPC, T, D], BF16, kind="ExternalInput")
    dram["Wq"] = nc.dram_tensor("Wq", [H, D, HD], BF16, kind="ExternalInput")
    dram["Wk"] = nc.dram_tensor("Wk", [H, D, HD], BF16, kind="ExternalInput")
    dram["Wv"] = nc.dram_tensor("Wv", [H, D, HD], BF16, kind="ExternalInput")
    dram["Wo"] = nc.dram_tensor("Wo", [H * HD, D], BF16, kind="ExternalInput")
    dram["bo"] = nc.dram_tensor("bo", [D], F32, kind="ExternalInput")
    dram["W1"] = nc.dram_tensor("W1", [D, F], BF16, kind="ExternalInput")
    dram["b1"] = nc.dram_tensor("b1", [F], F32, kind="ExternalInput")
    dram["W2"] = nc.dram_tensor("W2", [F, D], BF16, kind="ExternalInput")
    dram["b2"] = nc.dram_tensor("b2", [D], F32, kind="ExternalInput")
    dram["g1"] = nc.dram_tensor("g1", [D], F32, kind="ExternalInput")
    dram["be1"] = nc.dram_tensor("be1", [D], F32, kind="ExternalInput")
    dram["g2"] = nc.dram_tensor("g2", [D], F32, kind="ExternalInput")
    dram["be2"] = nc.dram_tensor("be2", [D], F32, kind="ExternalInput")
    out = nc.dram_tensor("out", [BPC, T, D], BF16, kind="ExternalOutput")

    ctx = dict(dram)
    ctx["nc"] = nc
    ctx["x_flat"] = dram["x"][:, :, :].flatten_outer_dims()
    ctx["out_flat"] = out[:, :, :].flatten_outer_dims()

    with tile.TileContext(nc) as tc, (
        tc.tile_pool(name="cst", bufs=1)
    ) as cst, (
        tc.tile_pool(name="small", bufs=6)
    ) as small, (
        tc.tile_pool(name="bcast", bufs=2)
    ) as bcast, (
        tc.tile_pool(name="ps", bufs=8, space="PSUM")
    ) as ps:
        ctx.update(tc=tc, cst=cst, small=small, bcast=bcast, ps=ps)

        ident = cst.tile([128, 128], BF16)
        make_identity(nc, ident)
        mask = cst.tile([128, 128], F32)
        nc.gpsimd.memset(mask, 0.0)
        nc.gpsimd.affine_select(
            out=mask, in_=mask, compare_op=ALU.is_ge, fill=NEG,
            base=0, pattern=[[-1, 128]], channel_multiplier=1,
        )
        eps_sb = cst.tile([128, 1], F32)
        nc.vector.memset(eps_sb, EPS)
        b1_sb = cst.tile([128, FT], F32)
        nc.sync.dma_start(out=b1_sb, in_=dram["b1"].rearrange("(f p) -> p f", p=128))
        ctx.update(ident=ident, mask=mask, eps_sb=eps_sb, b1_sb=b1_sb)

        with tc.tile_pool(name="ctxp", bufs=DT) as ctxp:
            ctx["ctxT"] = [
                ctxp.tile([128, NT], BF16, tag="ctx", name=f"ctxT{c}")
                for c in range(DT)
            ]
            _phase_ab(ctx, g1_id)

            with tc.tile_pool(name="x2p", bufs=RT) as x2p:
                ctx["x2"] = [
                    x2p.tile([128, D], F32, tag="x2p", name=f"x2_{c}")
                    for c in range(RT)
                ]
                _phase_wo(ctx, bo_zero)
                _phase_cde(
                    ctx, g2_id, b2_zero,
                    g1_id and g2_id and bo_zero and b2_zero,
                )

    _split_sync_waits(nc)
    return nc


def _f32(a):
    return np.ascontiguousarray(np.asarray(a), dtype=np.float32)


def _flags_from_inputs(inputs):
    return (
        bool(np.all(_f32(inputs["g1"]) == 1.0) and np.all(_f32(inputs["be1"]) == 0.0)),
        bool(np.all(_f32(inputs["g2"]) == 1.0) and np.all(_f32(inputs["be2"]) == 0.0)),
        bool(np.all(_f32(inputs["bo"]) == 0.0)),
        bool(np.all(_f32(inputs["b2"]) == 0.0)),
    )


def _to_bf16(a):
    import ml_dtypes

    return np.ascontiguousarray(np.asarray(a).astype(ml_dtypes.bfloat16))


class _AxonRunner:
    """Compile-once PJRT runner for the axon (remote NeuronCore) backend."""

    def __init__(self, flags):
        import jax
        from jax.sharding import Mesh, NamedSharding, PartitionSpec
        from concourse import bass2jax

        try:
            jax.config.update("jax_compilation_cache_dir", "/tmp/jax_comp_cache")
            jax.config.update("jax_persistent_cache_min_compile_time_secs", 10)
        except Exception:
            pass

        self.jax = jax

        bass2jax.install_neuronx_cc_hook()
        nc = self.nc = build_module(*flags)
        assert nc.dbg_addr is None, "debug build not supported in cached runner"
        partition_name = (
            nc.partition_id_tensor.name if nc.partition_id_tensor else None
        )

        in_names = []
        out_names = []
        out_avals = []
        for alloc in nc.m.functions[0].allocations:
            if not isinstance(alloc, mybir.MemoryLocationSet):
                continue
            name = alloc.memorylocations[0].name
            if alloc.kind == "ExternalInput":
                if name != partition_name:
                    in_names.append(name)
            elif alloc.kind == "ExternalOutput":
                out_names.append(name)
                out_avals.append(
                    jax.core.ShapedArray(
                        tuple(alloc.tensor_shape), mybir.dt.np(alloc.dtype)
                    )
                )
        self.in_names = in_names
        self.out_names = out_names
        n_params = len(in_names)
        n_outs = len(out_names)
        all_in_names = tuple(in_names + out_names)
        if partition_name is not None:
            all_in_names = all_in_names + (partition_name,)

        devices = jax.devices()[:N_CORES]
        assert len(devices) == N_CORES
        mesh = Mesh(np.asarray(devices), ("core",))
        self.mesh = mesh
        self.sh_core = NamedSharding(mesh, PartitionSpec("core"))
        self.sh_rep = NamedSharding(mesh, PartitionSpec(None))

        # x and the out-params are batch-sharded; weights are replicated.
        def spec_for(name):
            return PartitionSpec("core") if name in ("x", "out") else PartitionSpec(None)

        in_specs = tuple(spec_for(n) for n in in_names + out_names)
        out_specs = tuple(PartitionSpec("core") for _ in out_names)

        def _body(*args):
            operands = list(args)
            if partition_name is not None:
                operands.append(bass2jax.partition_id_tensor())
            outs = bass2jax._bass_exec_p.bind(
                *operands,
                out_avals=tuple(out_avals),
                in_names=all_in_names,
                out_names=tuple(out_names),
                lowering_input_output_aliases=(),
                sim_require_finite=True,
                sim_require_nnan=True,
                nc=nc,
            )
            return tuple(outs)

        from jax.experimental.shard_map import shard_map as _shard_map

        smap = _shard_map(
            _body, mesh=mesh, in_specs=in_specs, out_specs=out_specs,
            check_rep=False,
        )
        donate = tuple(range(n_params, n_params + n_outs))
        self.fn = jax.jit(smap, donate_argnums=donate, keep_unused=True)

        self._weight_key = None
        self._weight_dev = None
        self._donate_out = None

    def _weights_to_device(self, inputs):
        key = tuple(
            (id(inputs[k]), np.asarray(inputs[k]).shape) for k in WEIGHT_NAMES
        )
        if key == self._weight_key:
            return
        put = []
        for k in WEIGHT_NAMES:
            a = _to_bf16(inputs[k]) if k in BF16_WEIGHTS else _f32(inputs[k])
            put.append(self.jax.device_put(a, self.sh_rep))
        for a in put:
            a.block_until_ready()
        self._weight_dev = dict(zip(WEIGHT_NAMES, put))
        self._weight_key = key

    def _put_sharded_bf16(self, arr_f32):
        """Cast to bf16 and upload, one concurrent cast+put per device."""
        import concurrent.futures as cf

        import ml_dtypes

        jax = self.jax
        devs = list(self.mesh.devices.flat)
        n = len(devs)
        per = arr_f32.shape[0] // n

        def put(i):
            piece = np.ascontiguousarray(
                arr_f32[i * per : (i + 1) * per].astype(ml_dtypes.bfloat16)
            )
            return jax.device_put(piece, devs[i])

        with cf.ThreadPoolExecutor(n) as ex:
            bufs = list(ex.map(put, range(n)))
        return jax.make_array_from_single_device_arrays(
            (N_CORES * BPC, T, D), self.sh_core, bufs
        )

    def _get_sharded_f32(self, out):
        """Fetch shards concurrently, converting bf16->f32 in the fetch
        threads directly into a preallocated full-shape f32 array."""
        import concurrent.futures as cf

        shards = sorted(out.addressable_shards, key=lambda s: s.index)
        for s in shards:
            try:
                s.data.copy_to_host_async()
            except Exception:
                pass
        res = np.empty((B, T, D), np.float32)

        def fetch(i):
            lo = i * BPC
            buf = np.asarray(shards[i].data)  # bf16, host
            res[lo : lo + BPC] = buf.astype(np.float32)

        with cf.ThreadPoolExecutor(len(shards)) as ex:
            list(ex.map(fetch, range(len(shards))))
        return res

    def __call__(self, inputs):
        import time

        import ml_dtypes

        jax = self.jax
        prof = bool(os.environ.get("KERNEL_PROFILE"))
        threaded = os.environ.get("KERNEL_THREADED", "1") != "0"
        t0 = time.time()
        self._weights_to_device(inputs)
        t1 = time.time()
        x_np = _f32(inputs["x"])
        t2 = time.time()
        if threaded:
            xd = self._put_sharded_bf16(x_np)
        else:
            xd = jax.device_put(_to_bf16(x_np), self.sh_core)
        if prof:
            xd.block_until_ready()
        t3 = time.time()
        if self._donate_out is None:
            zeros = np.zeros((N_CORES * BPC, T, D), ml_dtypes.bfloat16)
            self._donate_out = jax.device_put(zeros, self.sh_core)
        args = []
        for n in self.in_names:
            if n == "x":
                args.append(xd)
            else:
                args.append(self._weight_dev[n])
        args.append(self._donate_out)
        # the donated buffer is consumed even if the call fails; drop the
        # reference first so a failure doesn't leave a dead array cached
        self._donate_out = None
        (out,) = self.fn(*args)
        if prof:
            out.block_until_ready()
        t4 = time.time()
        if threaded:
            final = self._get_sharded_f32(out)
        else:
            final = np.asarray(out).reshape(B, T, D).astype(np.float32)
        t5 = time.time()
        # out is fully written by the kernel; reuse it as the next donated
        # out-param so no fresh buffer ever needs uploading.
        self._donate_out = out
        if prof:
            print(
                f"[kprof] weights={t1-t0:.3f} cast={t2-t1:.3f} "
                f"upload={t3-t2:.3f} exec={t4-t3:.3f} fetch+conv={t5-t4:.3f} "
                f"total={t5-t0:.3f}",
                flush=True,
            )
        return final


_RUNNERS = {}
_NC_CACHE = {}


def _kernel_axon(inputs):
    flags = _flags_from_inputs(inputs)
    if flags not in _RUNNERS:
        _RUNNERS[flags] = _AxonRunner(flags)
    return _RUNNERS[flags](inputs)


_NATIVE_W_CACHE = {}


def _kernel_native(inputs):
    flags = _flags_from_inputs(inputs)
    if flags not in _NC_CACHE:
        _NC_CACHE[flags] = build_module(*flags)
    nc = _NC_CACHE[flags]

    xs = _to_bf16(inputs["x"])
    wkey = tuple(id(inputs[k]) for k in WEIGHT_NAMES)
    shared = _NATIVE_W_CACHE.get(wkey)
    if shared is None:
        shared = {
            k: (_to_bf16(inputs[k]) if k in BF16_WEIGHTS else _f32(inputs[k]))
            for k in WEIGHT_NAMES
        }
        _NATIVE_W_CACHE.clear()
        _NATIVE_W_CACHE[wkey] = shared
    in_maps = []
    for i in range(N_CORES):
        m = dict(shared)
        m["x"] = xs[i * BPC : (i + 1) * BPC]
        in_maps.append(m)

    res = run_bass_kernel_spmd(nc, in_maps, core_ids=list(range(N_CORES)))
    return np.concatenate(
        [res.results[i]["out"].astype(np.float32) for i in range(N_CORES)], axis=0
    )


def kernel(**inputs):
    if axon_active():
        try:
            return _kernel_axon(inputs)
        except Exception:
            import traceback

            traceback.print_exc()
            if os.environ.get("KERNEL_NO_FALLBACK"):
                raise
            return _kernel_native(inputs)
    return _kernel_native(inputs)
